# revision 36
# baseline (speedup 1.0000x reference)
"""Trainium2 Bass kernel for AdvancedPartialAttentionMasking (topk channel masking).

Math per (b, c): probs = softmax(x[b,c,:]) + 1e-6; H = -sum(probs*log(probs));
keep the 128 channels per sample with smallest H (ties -> lower channel index),
zero the rest.

Device formulation (per channel row, N = H*W = 3136, eps = 1e-6):
  e = exp(x);  Z = sum(e)  (7-chunk accumulation for accuracy)
  u = ln(N*e/Z + N*eps) = ln(N*q)           [q = softmax + eps]
  T = (1/N)*sum((e*N/Z)*u) + eps*sum(u)  ~= sum(q*u) = lnN*sum(q) - H + const
Ranking by T descending == ranking by H ascending (the constant drops out), and
keeping partial sums O(1) keeps fp32 error ~1e-7, far below ranking gaps.
Selection: rank_i = #{j: T_j > T_i} + #{j < i: T_j == T_i}; mask = rank < 128.
This reproduces jax.lax.top_k's stable tie-breaking exactly.

Sharding: pure data-parallel, 8 samples per core on 8 cores.
"""

import os
import sys

import numpy as np

if "/opt/trn_rl_repo" not in sys.path:
    sys.path.insert(0, "/opt/trn_rl_repo")

from contextlib import ExitStack  # noqa: E402

import concourse.bacc as bacc  # noqa: E402
import concourse.bass as bass  # noqa: E402
import concourse.bass_isa as bass_isa  # noqa: E402
import concourse.tile as tile  # noqa: E402
from concourse import mybir  # noqa: E402
from concourse.bass_utils import run_bass_kernel_spmd  # noqa: E402

F32 = mybir.dt.float32
ALU = mybir.AluOpType
AFT = mybir.ActivationFunctionType

B, C, H, W = 64, 256, 56, 56
N = H * W                      # 3136
NCORES = 8
BPC = B // NCORES              # samples per core = 8
NT = BPC * 2                   # [128]-channel tiles per core = 16
K = C // 2                     # 128 kept channels
EPS = 1e-6
NEPS = float(N) * EPS          # 0.003136
ZCHUNKS = 7
CHW = N // ZCHUNKS             # 448


def build_nc(nt=NT, zchunks=2, xbufs=6, ebufs=2, ubufs=2, sbufs=2,
             obufs=4, store_eng="gpsimd", small_eng="gpsimd", nswq=1,
             tdbg_on=False, scr_alias=True, par_reduce=True,
             ablate_rank=False, ablate_compute=False,
             newton=True, mask_eng="vector", zp_pair=False, repeat=1,
             zred="act"):
    """Build the per-core Bass program. nt must be even (2 tiles per sample)."""
    assert nt % 2 == 0
    assert N % zchunks == 0
    chw = N // zchunks
    nc = bacc.Bacc("TRN2", target_bir_lowering=False, num_swdge_queues=nswq)
    st_eng = getattr(nc, store_eng)
    sm_eng = getattr(nc, small_eng)

    x = nc.dram_tensor("x", [nt, 128, N], F32, kind="ExternalInput")
    tri = nc.dram_tensor("tri", [2, 128, C], F32, kind="ExternalInput")
    out = nc.dram_tensor("out", [nt, 128, N], F32, kind="ExternalOutput")
    tdbg = (
        nc.dram_tensor("tdbg", [nt, 128], F32, kind="ExternalOutput")
        if tdbg_on else None
    )
    diag = (
        nc.dram_tensor("diag", [2, 128, C], F32, kind="ExternalInput")
        if par_reduce else None
    )

    with tile.TileContext(nc) as tc, ExitStack() as ctx:
        xp = ctx.enter_context(tc.tile_pool(name="xp", bufs=xbufs))
        ep = ctx.enter_context(tc.tile_pool(name="ep", bufs=ebufs))
        up = ctx.enter_context(tc.tile_pool(name="up", bufs=ubufs))
        sp = ctx.enter_context(tc.tile_pool(name="sp", bufs=sbufs))
        op = ctx.enter_context(tc.tile_pool(name="op", bufs=obufs))
        sm = ctx.enter_context(tc.tile_pool(name="sm", bufs=4))
        tp = ctx.enter_context(tc.tile_pool(name="tp", bufs=4))
        bc = ctx.enter_context(tc.tile_pool(name="bc", bufs=2))
        ones = ctx.enter_context(tc.tile_pool(name="ones", bufs=1))
        dr = ctx.enter_context(tc.tile_pool(name="dr", bufs=2, space="DRAM"))

        tri_sb = [
            ones.tile([128, C], F32, tag=f"tri{p_}", name=f"tri{p_}")
            for p_ in range(2)
        ]
        for par in range(2):
            nc.sync.dma_start(out=tri_sb[par], in_=tri[par])
        neps_c = ones.tile([128, 1], F32, tag="neps", name="neps")
        nc.vector.memset(neps_c, NEPS)
        if par_reduce:
            diag_sb = [
                ones.tile([128, C], F32, tag=f"diag{p_}", name=f"diag{p_}")
                for p_ in range(2)
            ]
            for par in range(2):
                nc.sync.dma_start(out=diag_sb[par], in_=diag[par])
        if ablate_rank:
            mask_const = ones.tile([128, 1], F32, tag="maskc", name="maskc")
            nc.vector.memset(mask_const, 1.0)

        # per-sample state carried across the two channel-tiles
        stage = None      # DRAM [2,128] staging for this sample's T values
        tcols = [None, None]

        for t_rep in range(nt * repeat):
            t = t_rep % nt
            par = t % 2          # 0: channels 0-127, 1: channels 128-255
            if par == 0 and not par_reduce:
                stage = dr.tile([2, 128], F32, tag="stage")

            x_t = xp.tile([128, N], F32, tag="x")
            nc.sync.dma_start(out=x_t, in_=x[t])

            if ablate_compute:
                st_eng.dma_start(out=out[t], in_=x_t)
                prev_x = x_t
                continue

            # exp + chunked row sums
            e_t = ep.tile([128, N], F32, tag="e")
            u_t = up.tile([128, N], F32, tag="u")
            if zred == "ts":
                # single exp; Z summed by DVE tensor_scalar (2x mode) whose
                # elementwise output is discarded into u_t (overwritten by ln)
                zp = sm.tile([128, 1], F32, tag="zp")
                nc.scalar.activation(out=e_t, in_=x_t, func=AFT.Exp)
                nc.vector.tensor_scalar(
                    out=u_t, in0=e_t, scalar1=1.0, scalar2=None,
                    op0=ALU.mult, op1=ALU.add, accum_out=zp,
                )
                zchunks_eff = 1
            elif zred == "dve":
                nzc = max(zchunks, 2)
                zp = sm.tile([128, nzc], F32, tag="zp")
                nc.scalar.activation(out=e_t, in_=x_t, func=AFT.Exp)
                nc.vector.tensor_reduce(
                    out=zp[:, 0:nzc],
                    in_=e_t[:].rearrange("p (c f) -> p c f", c=nzc),
                    axis=mybir.AxisListType.X, op=ALU.add,
                )
                zchunks_eff = nzc
            else:
                zp = sm.tile([128, 8], F32, tag="zp")
                for cth in range(zchunks):
                    sl = slice(cth * chw, (cth + 1) * chw)
                    nc.scalar.activation(
                        out=e_t[:, sl], in_=x_t[:, sl], func=AFT.Exp,
                        accum_out=zp[:, cth:cth + 1],
                    )
                zchunks_eff = zchunks
            # z_n = (sum of chunk sums) / N, then ninvz = 1/z_n = N/Z
            z = sm.tile([128, 1], F32, tag="z")
            nc.vector.tensor_reduce(
                out=z, in_=zp[:, 0:zchunks_eff], axis=mybir.AxisListType.X,
                op=ALU.add,
            )
            zn = sm.tile([128, 1], F32, tag="zn")
            nc.vector.tensor_scalar(
                out=zn, in0=z, scalar1=1.0 / N, scalar2=None, op0=ALU.mult
            )
            if newton:
                r0 = sm.tile([128, 1], F32, tag="r0")
                nc.vector.reciprocal(out=r0, in_=zn)
                cc = sm.tile([128, 1], F32, tag="cc")
                nc.vector.tensor_mul(cc, zn, r0)
                c2 = sm.tile([128, 1], F32, tag="c2")
                nc.vector.tensor_scalar(
                    out=c2, in0=cc, scalar1=-1.0, scalar2=2.0,
                    op0=ALU.mult, op1=ALU.add,
                )
                ninvz = sm.tile([128, 1], F32, tag="ninvz")
                nc.vector.tensor_mul(ninvz, r0, c2)
            else:
                ninvz = sm.tile([128, 1], F32, tag="ninvz")
                nc.vector.reciprocal(out=ninvz, in_=zn)

            # u = ln(e * (N/Z) + N*eps), accumulate sum(u)
            su = sm.tile([128, 1], F32, tag="su")
            nc.scalar.activation(
                out=u_t, in_=e_t, func=AFT.Ln, bias=neps_c, scale=ninvz,
                accum_out=su,
            )
            # A = sum((e * N/Z) * u)
            scr = e_t if scr_alias else sp.tile([128, N], F32, tag="scr")
            a = sm.tile([128, 1], F32, tag="a")
            nc.vector.scalar_tensor_tensor(
                out=scr, in0=e_t, scalar=ninvz, in1=u_t,
                op0=ALU.mult, op1=ALU.mult, accum_out=a,
            )
            # T = A/N + eps*su
            tmp = sm.tile([128, 1], F32, tag="tmp")
            nc.vector.tensor_scalar(
                out=tmp, in0=su, scalar1=EPS, scalar2=None, op0=ALU.mult
            )
            t_col = tp.tile([128, 1], F32, tag="tcol")
            nc.vector.scalar_tensor_tensor(
                out=t_col, in0=a, scalar=1.0 / N, in1=tmp,
                op0=ALU.mult, op1=ALU.add,
            )
            tcols[par] = t_col

            if ablate_rank:
                o_t = op.tile([128, N], F32, tag="o")
                nc.vector.tensor_scalar(
                    out=o_t, in0=x_t, scalar1=mask_const, scalar2=None, op0=ALU.mult
                )
                st_eng.dma_start(out=out[t], in_=o_t)
                prev_x = x_t
                continue

            if not par_reduce:
                # stage T into DRAM row `par` (128 x 4B descriptors)
                stage_row = bass.AP(
                    tensor=stage.tensor,
                    offset=stage[:].offset + par * 128,
                    ap=[[1, 128], [0, 1]],
                )
                sm_eng.dma_start(out=stage_row, in_=t_col)
            if tdbg_on:
                tdbg_row = bass.AP(
                    tensor=tdbg, offset=t * 128, ap=[[1, 128], [0, 1]]
                )
                sm_eng.dma_start(out=tdbg_row, in_=t_col)

            if par == 1:
                # broadcast this sample's 256 T values across 128 partitions
                s_bc = bc.tile([128, C], F32, tag="sbc")
                if par_reduce:
                    m1 = bc.tile([128, C], F32, tag="m1")
                    nc.vector.tensor_scalar(
                        out=m1, in0=diag_sb[0], scalar1=tcols[0], scalar2=None,
                        op0=ALU.mult,
                    )
                    m2 = bc.tile([128, C], F32, tag="m2")
                    nc.vector.scalar_tensor_tensor(
                        out=m2, in0=diag_sb[1], scalar=tcols[1], in1=m1,
                        op0=ALU.mult, op1=ALU.add,
                    )
                    nc.gpsimd.partition_all_reduce(
                        out_ap=s_bc[:], in_ap=m2[:], channels=128,
                        reduce_op=bass_isa.ReduceOp.add,
                    )
                else:
                    bc_src = bass.AP(
                        tensor=stage.tensor, offset=stage[:].offset,
                        ap=[[0, 128], [1, C]],
                    )
                    sm_eng.dma_start(out=s_bc, in_=bc_src)

                for par2 in range(2):
                    tt = t - 1 + par2
                    t_c = tcols[par2]
                    ngt = tp.tile([128, 1], F32, tag="ngt")
                    scr256 = sm.tile([128, C], F32, tag="scr256")
                    nc.vector.tensor_scalar(
                        out=scr256, in0=s_bc, scalar1=t_c, scalar2=None,
                        op0=ALU.is_gt, op1=ALU.add, accum_out=ngt,
                    )
                    neq = tp.tile([128, 1], F32, tag="neq")
                    scr256b = sm.tile([128, C], F32, tag="scr256b")
                    nc.vector.scalar_tensor_tensor(
                        out=scr256b, in0=s_bc, scalar=t_c, in1=tri_sb[par2],
                        op0=ALU.is_equal, op1=ALU.mult, accum_out=neq,
                    )
                    rank = tp.tile([128, 1], F32, tag="rank")
                    nc.vector.tensor_add(rank, ngt, neq)
                    mask01 = tp.tile([128, 1], F32, tag="mask")
                    nc.vector.tensor_scalar(
                        out=mask01, in0=rank, scalar1=float(K) - 0.5, scalar2=None,
                        op0=ALU.is_lt,
                    )
                    # out = x * mask  (per-partition scalar broadcast, 2x mode)
                    xx = x_t if par2 == 1 else prev_x
                    o_t = op.tile([128, N], F32, tag="o")
                    if mask_eng == "scalar":
                        nc.scalar.activation(
                            out=o_t, in_=xx, func=AFT.Copy, scale=mask01, bias=0.0
                        )
                    else:
                        getattr(nc, mask_eng).tensor_scalar(
                            out=o_t, in0=xx, scalar1=mask01, scalar2=None,
                            op0=ALU.mult,
                        )
                    st_eng.dma_start(out=out[tt], in_=o_t)
            prev_x = x_t

    nc.finalize()
    return nc


_TRI = None


def _tri_const():
    global _TRI
    if _TRI is None:
        tri = np.zeros((2, 128, C), np.float32)
        for par in range(2):
            i = np.arange(128)[:, None] + par * 128
            j = np.arange(C)[None, :]
            tri[par] = (j < i).astype(np.float32)
        _TRI = tri
    return _TRI


_DIAG = None


def _diag_const():
    global _DIAG
    if _DIAG is None:
        d = np.zeros((2, 128, C), np.float32)
        for par in range(2):
            i = np.arange(128)[:, None] + par * 128
            j = np.arange(C)[None, :]
            d[par] = (j == i).astype(np.float32)
        _DIAG = d
    return _DIAG


def input_names(nc):
    import concourse.mybir as mb
    names = set()
    for alloc in nc.m.functions[0].allocations:
        if isinstance(alloc, mb.MemoryLocationSet) and alloc.kind == "ExternalInput":
            names.add(alloc.memorylocations[0].name)
    return names


def make_in_maps(nc, x):
    xs = np.ascontiguousarray(x).reshape(NCORES, NT, 128, N)
    avail = {"tri": _tri_const(), "diag": _diag_const()}
    names = input_names(nc)
    return [
        {"x": xs[i], **{k: v for k, v in avail.items() if k in names}}
        for i in range(NCORES)
    ]


_NC = None


def kernel(x: np.ndarray) -> np.ndarray:
    global _NC
    assert x.shape == (B, C, H, W) and x.dtype == np.float32
    if _NC is None:
        _NC = build_nc()
    in_maps = make_in_maps(_NC, x)
    res = run_bass_kernel_spmd(_NC, in_maps, core_ids=list(range(NCORES)))
    outs = [res.results[i]["out"] for i in range(NCORES)]
    return np.concatenate(outs, axis=0).reshape(B, C, H, W)


if __name__ == "__main__":
    xr = np.random.default_rng(0).standard_normal((B, C, H, W), dtype=np.float32)
    y = kernel(xr)
    print("ok", y.shape, y.dtype, float(np.abs(y).sum()))
